# revision 13
# baseline (speedup 1.0000x reference)
"""Multi-head attention (RoPE) Trainium2 Bass kernel.

Problem: B=2, S=2048, d_model=1024, 16 heads x head_dim 64, fp32.

The reference faithfully replicates a torch rank-5 reshape bug: the
attention output [1,H,B,S,D] is transposed to [1,H,B,S,D]->(0,2,1,3,4)
and flat-reshaped to [B,S,H*D] BEFORE the Wo projection. Net semantics:
  out[b2, s2, :] = flatten(O[b, h, s0:s0+16, :]) @ Wo + bo
  with h = b2*8 + s2//256, b = (s2//128)%2, s0 = (s2%128)*16,
so the projection is PER-HEAD (contraction mixes 16 seq x 64 dims of one
head) and every (b,h) yields an independent [128, 1024] output block.

Sharding (8 cores): batch (2) x head groups (4 groups of 4 heads).
Per core: QKV slices via f32r matmuls in transposed layout, RoPE
(rotate-half via a signed permutation matmul), per-head attention with
unnormalized softmax (ones-column appended to V gives the denominator),
normalize into ot64 [64, 4head, S], then per-head scrambled projection
against full Wo. Host places the 32 independent blocks and adds bo.
"""

import numpy as np

import concourse.bass as bass
import concourse.tile as tile
from concourse import bacc, mybir
from concourse import bass_utils

F32 = mybir.dt.float32
MM_DT = mybir.dt.float32r  # matmul operand dtype (float32r: 1 cyc/row)

B, S, DM, H, HD = 2, 2048, 1024, 16, 64
N_CORES = 8
HG = 4          # head groups (tensor-parallel factor)
GD = DM // HG   # qkv dims per core = 256
NKC = DM // 128   # d_model contraction chunks = 8
NST = S // 512    # seq tiles of 512 = 4
NSK = S // 128    # seq_k chunks of 128 = 16
NQP = S // 1024   # seq_q pairs of 1024 = 2


def _emit(nc, tc, ap, debug=False):
    import contextlib

    ctx = contextlib.ExitStack()
    with ctx:
        consts = ctx.enter_context(tc.tile_pool(name="consts", bufs=1))
        big = ctx.enter_context(tc.tile_pool(name="big", bufs=1))

        # ---- constants / weights to SBUF ----
        cosb = consts.tile([128, S], F32)
        nc.sync.dma_start(cosb, ap["cosb"])
        sinb = consts.tile([128, S], F32)
        nc.sync.dma_start(sinb, ap["sinb"])
        rot = consts.tile([128, 128], MM_DT)
        nc.sync.dma_start(rot, ap["rot"].bitcast(MM_DT))
        bqc = consts.tile([128, 2], F32)
        nc.gpsimd.dma_start(bqc, ap["bq2"].rearrange("c p -> p c"))
        bkc = consts.tile([128, 2], F32)
        nc.gpsimd.dma_start(bkc, ap["bk2"].rearrange("c p -> p c"))
        bvb = consts.tile([128, GD], F32)
        nc.gpsimd.dma_start(bvb, ap["bv"].partition_broadcast(128))
        onescol = consts.tile([1, 64], F32)
        nc.vector.memset(onescol, 1.0)

        wq = consts.tile([128, NKC, GD], MM_DT)
        nc.sync.dma_start(wq, ap["wq"].rearrange("(kc p) m -> p kc m", p=128).bitcast(MM_DT))
        wk = consts.tile([128, NKC, GD], MM_DT)
        nc.sync.dma_start(wk, ap["wk"].rearrange("(kc p) m -> p kc m", p=128).bitcast(MM_DT))
        wv = consts.tile([128, NKC, GD], MM_DT)
        nc.sync.dma_start(wv, ap["wv"].rearrange("(kc p) m -> p kc m", p=128).bitcast(MM_DT))

        # ---- persistent activation buffers ----
        qe = [big.tile([128, S], MM_DT, name=f"qe{mc}", tag=f"qe{mc}") for mc in range(2)]
        ke = [big.tile([128, S], MM_DT, name=f"ke{mc}", tag=f"ke{mc}") for mc in range(2)]
        # V natural layout + ones column: [128 seq, kc, head, 65]
        vsb = big.tile([128, NSK, 4, 65], MM_DT, name="vsb", tag="vsb")
        nc.vector.memset(vsb[:, :, :, 64:65].bitcast(F32), 1.0)
        # normalized attention output, heads on the free axis: [64, head, S]
        ot64 = big.tile([64, 4, S], MM_DT, name="ot64", tag="ot64")

        # ================= Phase B: QKV projections + RoPE =================
        with (
            tc.tile_pool(name="xt", bufs=6) as xt_pool,
            tc.tile_pool(name="raw", bufs=3) as raw_pool,
            tc.tile_pool(name="t1", bufs=3) as t1_pool,
            tc.tile_pool(name="ps_qk", bufs=1, space="PSUM") as ps_qk,
            tc.tile_pool(name="ps_v", bufs=1, space="PSUM") as ps_v,
        ):
            for st in range(NST):
                sl = slice(st * 512, (st + 1) * 512)
                pqk = {}
                pv = {}
                for tgt in range(2):
                    for mc in range(2):
                        pqk[tgt, mc] = ps_qk.tile(
                            [128, 512], F32, name=f"pqk{tgt}{mc}", tag=f"qk{tgt}{mc}"
                        )
                for ss in range(4):
                    pv[ss] = ps_v.tile([128, GD], F32, name=f"pv{ss}", tag=f"v{ss}")
                for kc in range(NKC):
                    xt_kc = xt_pool.tile([128, 512], MM_DT)
                    nc.sync.dma_start(
                        xt_kc,
                        ap["xt"][kc * 128:(kc + 1) * 128, sl].bitcast(MM_DT),
                    )
                    for tgt in range(2):
                        w_sb = wq if tgt == 0 else wk
                        for mc in range(2):
                            nc.tensor.matmul(
                                pqk[tgt, mc],
                                lhsT=w_sb[:, kc, mc * 128:(mc + 1) * 128],
                                rhs=xt_kc,
                                start=(kc == 0),
                                stop=(kc == NKC - 1),
                            )
                    for ss in range(4):
                        nc.tensor.matmul(
                            pv[ss],
                            lhsT=xt_kc[:, ss * 128:(ss + 1) * 128],
                            rhs=wv[:, kc, :],
                            start=(kc == 0),
                            stop=(kc == NKC - 1),
                        )
                for tgt in range(2):
                    bias = bqc if tgt == 0 else bkc
                    dst = qe if tgt == 0 else ke
                    for mc in range(2):
                        raw = raw_pool.tile([128, 512], MM_DT)
                        nc.vector.tensor_scalar_add(raw, pqk[tgt, mc], bias[:, mc:mc + 1])
                        rps = ps_qk.tile(
                            [128, 512], F32, name=f"rps{tgt}{mc}", tag=f"qk{tgt}{mc}"
                        )
                        nc.tensor.matmul(rps, lhsT=rot, rhs=raw, start=True, stop=True)
                        t1 = t1_pool.tile([128, 512], F32)
                        nc.vector.tensor_mul(t1, rps, sinb[:, sl])
                        d = dst[mc][:, sl]
                        nc.vector.tensor_mul(d, raw, cosb[:, sl])
                        nc.vector.tensor_add(d, d, t1)
                for ss in range(4):
                    nc.vector.tensor_add(
                        vsb[:, st * 4 + ss, :, 0:64],
                        pv[ss].rearrange("p (h d) -> p h d", h=4),
                        bvb.rearrange("p (h d) -> p h d", h=4),
                    )

        if debug:
            for mc in range(2):
                nc.sync.dma_start(ap["qe_dbg"][mc], qe[mc].bitcast(F32))
                nc.sync.dma_start(ap["ke_dbg"][mc], ke[mc].bitcast(F32))
            nc.sync.dma_start(ap["v_dbg"], vsb.bitcast(F32))

        # ================= Phase C: attention =================
        with (
            tc.tile_pool(name="e", bufs=2) as e_pool,
            tc.tile_pool(name="rcp", bufs=4) as rcp_pool,
            tc.tile_pool(name="ps_s", bufs=1, space="PSUM") as ps_s,
            tc.tile_pool(name="ps_u", bufs=1, space="PSUM") as ps_u,
        ):
            for hc in range(2):
                for qp in range(NQP):
                    u = [ps_u.tile([65, 512], F32, name=f"u{i}", tag=f"u{i}") for i in range(4)]
                    for kc in range(NSK):
                        for hi in range(2):
                            hpart = slice(hi * 64, (hi + 1) * 64)
                            g = ps_s.tile([128, 1024], F32, tag=f"sg{hi}", name=f"sg{hi}")
                            for qh in range(2):
                                nc.tensor.matmul(
                                    g[:, qh * 512:(qh + 1) * 512],
                                    lhsT=ke[hc][hpart, kc * 128:(kc + 1) * 128],
                                    rhs=qe[hc][hpart, (qp * 2 + qh) * 512:(qp * 2 + qh + 1) * 512],
                                    start=True,
                                    stop=True,
                                )
                            e = e_pool.tile([128, 1024], MM_DT)
                            nc.scalar.activation(
                                e, g, mybir.ActivationFunctionType.Exp, scale=0.125
                            )
                            for qh in range(2):
                                nc.tensor.matmul(
                                    u[hi * 2 + qh],
                                    lhsT=vsb[:, kc, hc * 2 + hi, :],
                                    rhs=e[:, qh * 512:(qh + 1) * 512],
                                    start=(kc == 0),
                                    stop=(kc == NSK - 1),
                                )
                    for hi in range(2):
                        for qh in range(2):
                            uu = u[hi * 2 + qh]
                            r = rcp_pool.tile([1, 512], F32)
                            nc.vector.reciprocal(r, uu[64:65, :])
                            dbc = ps_s.tile([64, 512], F32, tag="sg0", name="dbc")
                            nc.tensor.matmul(dbc, lhsT=onescol, rhs=r, start=True, stop=True)
                            dbc_sb = rcp_pool.tile([64, 512], F32, tag="dbc_sb")
                            nc.vector.tensor_copy(dbc_sb, dbc)
                            osl = slice((qp * 2 + qh) * 512, (qp * 2 + qh + 1) * 512)
                            nc.vector.tensor_mul(
                                ot64[:, hc * 2 + hi, osl], uu[0:64, :], dbc_sb
                            )

        if debug:
            nc.sync.dma_start(ap["ot_dbg"], ot64.bitcast(F32))

        # ====== Phase D: per-head scrambled output projection (full Wo) ======
        # out block rows mix 16 seq x 64 dim: row s2r contracts
        # O[s2r*16 + j, d] * Wo[j*64 + d, :].  rhs per j: strided view of ot64.
        ot_r = ot64.rearrange("p h (s2r g) -> p h s2r g", g=16)
        with (
            tc.tile_pool(name="wo_mc", bufs=2) as wo_pool,
            tc.tile_pool(name="ysb", bufs=3) as y_pool,
            tc.tile_pool(name="ps_y", bufs=3, space="PSUM") as ps_y,
        ):
            for mc in range(NKC):
                wo_mc = wo_pool.tile([64, 16, 128], MM_DT)
                nc.sync.dma_start(
                    wo_mc,
                    ap["wo"][:, mc * 128:(mc + 1) * 128]
                    .rearrange("(j p) m -> p j m", p=64)
                    .bitcast(MM_DT),
                )
                py = ps_y.tile([128, 512], F32)
                for j in range(16):
                    nc.tensor.matmul(
                        py,
                        lhsT=wo_mc[:, j, :],
                        rhs=ot_r[:, :, :, j],
                        start=(j == 0),
                        stop=(j == 15),
                    )
                ysb = y_pool.tile([128, 512], F32)
                nc.vector.tensor_copy(ysb, py)
                nc.sync.dma_start(ap["ypt"][mc * 128:(mc + 1) * 128, :], ysb)


def _build(debug=False):
    nc = bacc.Bacc("TRN2", target_bir_lowering=False, debug=False, num_devices=N_CORES)
    ap = {}
    ap["xt"] = nc.dram_tensor("xt", [DM, S], F32, kind="ExternalInput").ap()
    ap["wq"] = nc.dram_tensor("wq", [DM, GD], F32, kind="ExternalInput").ap()
    ap["wk"] = nc.dram_tensor("wk", [DM, GD], F32, kind="ExternalInput").ap()
    ap["wv"] = nc.dram_tensor("wv", [DM, GD], F32, kind="ExternalInput").ap()
    ap["wo"] = nc.dram_tensor("wo", [DM, DM], F32, kind="ExternalInput").ap()
    ap["bq2"] = nc.dram_tensor("bq2", [2, 128], F32, kind="ExternalInput").ap()
    ap["bk2"] = nc.dram_tensor("bk2", [2, 128], F32, kind="ExternalInput").ap()
    ap["bv"] = nc.dram_tensor("bv", [GD], F32, kind="ExternalInput").ap()
    ap["cosb"] = nc.dram_tensor("cosb", [128, S], F32, kind="ExternalInput").ap()
    ap["sinb"] = nc.dram_tensor("sinb", [128, S], F32, kind="ExternalInput").ap()
    ap["rot"] = nc.dram_tensor("rot", [128, 128], F32, kind="ExternalInput").ap()
    # per-core output: Y^T [1024, 512] (columns = 4 heads x 128 block rows)
    ap["ypt"] = nc.dram_tensor("ypt", [DM, 512], F32, kind="ExternalOutput").ap()
    if debug:
        ap["qe_dbg"] = nc.dram_tensor("qe_dbg", [2, 128, S], F32, kind="ExternalOutput").ap()
        ap["ke_dbg"] = nc.dram_tensor("ke_dbg", [2, 128, S], F32, kind="ExternalOutput").ap()
        ap["v_dbg"] = nc.dram_tensor("v_dbg", [128, NSK, 4, 65], F32, kind="ExternalOutput").ap()
        ap["ot_dbg"] = nc.dram_tensor("ot_dbg", [64, 4, S], F32, kind="ExternalOutput").ap()

    with tile.TileContext(nc) as tc:
        _emit(nc, tc, ap, debug=debug)
    nc.compile()
    return nc


_CACHE = {}


def _rope_tables():
    inv_freq = (1.0 / (10000.0 ** (np.arange(0, HD, 2, dtype=np.float32) / HD))).astype(np.float32)
    t = np.arange(S, dtype=np.float32)
    freqs = np.outer(t, inv_freq).astype(np.float32)  # [S, 32]
    emb = np.concatenate([freqs, freqs], axis=-1)  # [S, 64]
    cosT = np.cos(emb).astype(np.float32).T  # [64, S]
    sinT = np.sin(emb).astype(np.float32).T
    cosb = np.ascontiguousarray(np.concatenate([cosT, cosT], axis=0))  # [128, S]
    sinb = np.ascontiguousarray(np.concatenate([sinT, sinT], axis=0))
    return cosb, sinb


def _rot_matrix():
    p64 = np.zeros((HD, HD), dtype=np.float32)
    for i in range(32):
        p64[i, i + 32] = -1.0
        p64[i + 32, i] = 1.0
    p = np.zeros((128, 128), dtype=np.float32)
    p[0:64, 0:64] = p64
    p[64:128, 64:128] = p64
    return np.ascontiguousarray(p.T)  # lhsT = P^T


def kernel(x, Wq, bq, Wk, bk, Wv, bv, Wo, bo):
    x = np.asarray(x, dtype=np.float32)
    Wq, bq = np.asarray(Wq, np.float32), np.asarray(bq, np.float32)
    Wk, bk = np.asarray(Wk, np.float32), np.asarray(bk, np.float32)
    Wv, bv = np.asarray(Wv, np.float32), np.asarray(bv, np.float32)
    Wo, bo = np.asarray(Wo, np.float32), np.asarray(bo, np.float32)

    if "nc" not in _CACHE:
        _CACHE["nc"] = _build()
    nc = _CACHE["nc"]

    cosb, sinb = _rope_tables()
    rot = _rot_matrix()
    xt_b = [np.ascontiguousarray(x[b].T) for b in range(B)]  # [DM, S]
    wo_c = np.ascontiguousarray(Wo)

    in_maps = []
    for c in range(N_CORES):
        b, hg = divmod(c, HG)
        sl = slice(hg * GD, (hg + 1) * GD)
        in_maps.append(
            {
                "xt": xt_b[b],
                "wq": np.ascontiguousarray(Wq[:, sl]),
                "wk": np.ascontiguousarray(Wk[:, sl]),
                "wv": np.ascontiguousarray(Wv[:, sl]),
                "wo": wo_c,
                "bq2": np.ascontiguousarray(bq[sl].reshape(2, 128)),
                "bk2": np.ascontiguousarray(bk[sl].reshape(2, 128)),
                "bv": np.ascontiguousarray(bv[sl]),
                "cosb": cosb,
                "sinb": sinb,
                "rot": rot,
            }
        )

    res = bass_utils.run_bass_kernel_spmd(nc, in_maps, core_ids=list(range(N_CORES)))
    _CACHE["last_results"] = res

    # Block placement: core (b, hg), local head hl -> global head h = hg*4+hl,
    # lands at out[h//8, (h%8)*256 + b*128 : +128, :].
    out = np.empty((B, S, DM), dtype=np.float32)
    for c in range(N_CORES):
        b, hg = divmod(c, HG)
        ypt = res.results[c]["ypt"]  # [1024, 512]
        for hl in range(4):
            h = hg * 4 + hl
            b2 = h // 8
            s2 = (h % 8) * 256 + b * 128
            out[b2, s2:s2 + 128, :] = ypt[:, hl * 128:(hl + 1) * 128].T
    out += bo[None, None, :]
    return out


# revision 18
# speedup vs baseline: 1.2734x; 1.2734x over previous
"""Multi-head attention (RoPE) Trainium2 Bass kernel.

Problem: B=2, S=2048, d_model=1024, 16 heads x head_dim 64, fp32.

The reference faithfully replicates a torch rank-5 reshape bug: the
attention output [1,H,B,S,D] is transposed to [1,H,B,S,D]->(0,2,1,3,4)
and flat-reshaped to [B,S,H*D] BEFORE the Wo projection. Net semantics:
  out[b2, s2, :] = flatten(O[b, h, s0:s0+16, :]) @ Wo + bo
  with h = b2*8 + s2//256, b = (s2//128)%2, s0 = (s2%128)*16,
so the projection is PER-HEAD (contraction mixes 16 seq x 64 dims of one
head) and every (b,h) yields an independent [128, 1024] output block.

Sharding (8 cores): batch (2) x head groups (4 groups of 4 heads).
Per core: QKV slices via f32r matmuls in transposed layout, RoPE
(rotate-half via a signed permutation matmul), per-head attention with
unnormalized softmax (ones-column appended to V gives the denominator),
normalize into ot64 [64, 4head, S], then per-head scrambled projection
against full Wo. Host places the 32 independent blocks and adds bo.
"""

import numpy as np

import concourse.bass as bass
import concourse.tile as tile
from concourse import bacc, mybir
from concourse import bass_utils

F32 = mybir.dt.float32
MM_DT = mybir.dt.float32r  # matmul operand dtype (float32r: 1 cyc/row)

B, S, DM, H, HD = 2, 2048, 1024, 16, 64
N_CORES = 8
HG = 4          # head groups (tensor-parallel factor)
GD = DM // HG   # qkv dims per core = 256
NKC = DM // 128   # d_model contraction chunks = 8
NST = S // 512    # seq tiles of 512 = 4
NSK = S // 128    # seq_k chunks of 128 = 16
NQP = S // 1024   # seq_q pairs of 1024 = 2


def _emit(nc, tc, ap, debug=False):
    import contextlib

    ctx = contextlib.ExitStack()
    with ctx:
        consts = ctx.enter_context(tc.tile_pool(name="consts", bufs=1))
        big = ctx.enter_context(tc.tile_pool(name="big", bufs=1))

        # ---- constants / weights to SBUF ----
        cosb = consts.tile([128, S], F32)
        nc.sync.dma_start(cosb, ap["cosb"])
        sinb = consts.tile([128, S], F32)
        nc.sync.dma_start(sinb, ap["sinb"])
        rot = consts.tile([128, 128], MM_DT)
        nc.sync.dma_start(rot, ap["rot"].bitcast(MM_DT))
        bqc = consts.tile([128, 2], F32)
        nc.gpsimd.dma_start(bqc, ap["bq2"].rearrange("c p -> p c"))
        bkc = consts.tile([128, 2], F32)
        nc.gpsimd.dma_start(bkc, ap["bk2"].rearrange("c p -> p c"))
        bvb = consts.tile([128, GD], F32)
        nc.gpsimd.dma_start(bvb, ap["bv"].partition_broadcast(128))

        wq = consts.tile([128, NKC, GD], MM_DT)
        nc.sync.dma_start(wq, ap["wq"].rearrange("(kc p) m -> p kc m", p=128).bitcast(MM_DT))
        wk = consts.tile([128, NKC, GD], MM_DT)
        nc.sync.dma_start(wk, ap["wk"].rearrange("(kc p) m -> p kc m", p=128).bitcast(MM_DT))
        wv = consts.tile([128, NKC, GD], MM_DT)
        nc.sync.dma_start(wv, ap["wv"].rearrange("(kc p) m -> p kc m", p=128).bitcast(MM_DT))

        # ---- persistent activation buffers ----
        qe = [big.tile([128, S], MM_DT, name=f"qe{mc}", tag=f"qe{mc}") for mc in range(2)]
        ke = [big.tile([128, S], MM_DT, name=f"ke{mc}", tag=f"ke{mc}") for mc in range(2)]
        # V natural layout + ones column: [128 seq, kc, head, 65]
        vsb = big.tile([128, NSK, 4, 65], MM_DT, name="vsb", tag="vsb")
        nc.vector.memset(vsb[:, :, :, 64:65].bitcast(F32), 1.0)
        # normalized attention output, heads on the free axis: [64, head, S]
        ot64 = big.tile([64, 4, S], MM_DT, name="ot64", tag="ot64")

        # ================= Phase B: QKV projections + RoPE =================
        with (
            tc.tile_pool(name="xt", bufs=6) as xt_pool,
            tc.tile_pool(name="raw", bufs=3) as raw_pool,
            tc.tile_pool(name="t1", bufs=3) as t1_pool,
            tc.tile_pool(name="ps_qk", bufs=1, space="PSUM") as ps_qk,
            tc.tile_pool(name="ps_v", bufs=1, space="PSUM") as ps_v,
        ):
            for st in range(NST):
                sl = slice(st * 512, (st + 1) * 512)
                pqk = {}
                pv = {}
                for tgt in range(2):
                    for mc in range(2):
                        pqk[tgt, mc] = ps_qk.tile(
                            [128, 512], F32, name=f"pqk{tgt}{mc}", tag=f"qk{tgt}{mc}"
                        )
                for ss in range(4):
                    pv[ss] = ps_v.tile([128, GD], F32, name=f"pv{ss}", tag=f"v{ss}")
                for kc in range(NKC):
                    xt_kc = xt_pool.tile([128, 512], MM_DT)
                    nc.sync.dma_start(
                        xt_kc,
                        ap["xt"][kc * 128:(kc + 1) * 128, sl].bitcast(MM_DT),
                    )
                    for tgt in range(2):
                        w_sb = wq if tgt == 0 else wk
                        for mc in range(2):
                            nc.tensor.matmul(
                                pqk[tgt, mc],
                                lhsT=w_sb[:, kc, mc * 128:(mc + 1) * 128],
                                rhs=xt_kc,
                                start=(kc == 0),
                                stop=(kc == NKC - 1),
                            )
                    for ss in range(4):
                        nc.tensor.matmul(
                            pv[ss],
                            lhsT=xt_kc[:, ss * 128:(ss + 1) * 128],
                            rhs=wv[:, kc, :],
                            start=(kc == 0),
                            stop=(kc == NKC - 1),
                        )
                for tgt in range(2):
                    bias = bqc if tgt == 0 else bkc
                    dst = qe if tgt == 0 else ke
                    for mc in range(2):
                        raw = raw_pool.tile([128, 512], MM_DT)
                        nc.vector.tensor_scalar_add(raw, pqk[tgt, mc], bias[:, mc:mc + 1])
                        rps = ps_qk.tile(
                            [128, 512], F32, name=f"rps{tgt}{mc}", tag=f"qk{tgt}{mc}"
                        )
                        nc.tensor.matmul(rps, lhsT=rot, rhs=raw, start=True, stop=True)
                        t1 = t1_pool.tile([128, 512], F32)
                        nc.vector.tensor_mul(t1, rps, sinb[:, sl])
                        d = dst[mc][:, sl]
                        nc.vector.tensor_mul(d, raw, cosb[:, sl])
                        nc.vector.tensor_add(d, d, t1)
                for ss in range(4):
                    nc.vector.tensor_add(
                        vsb[:, st * 4 + ss, :, 0:64],
                        pv[ss].rearrange("p (h d) -> p h d", h=4),
                        bvb.rearrange("p (h d) -> p h d", h=4),
                    )

        if debug:
            for mc in range(2):
                nc.sync.dma_start(ap["qe_dbg"][mc], qe[mc].bitcast(F32))
                nc.sync.dma_start(ap["ke_dbg"][mc], ke[mc].bitcast(F32))
            nc.sync.dma_start(ap["v_dbg"], vsb.bitcast(F32))

        # ================= Phase C: attention =================
        with (
            tc.tile_pool(name="e", bufs=3) as e_pool,
            tc.tile_pool(name="rcp", bufs=4) as rcp_pool,
            tc.tile_pool(name="rdram", bufs=4, space="DRAM") as rdram_pool,
            tc.tile_pool(name="ps_s", bufs=1, space="PSUM") as ps_s,
            tc.tile_pool(name="ps_u", bufs=1, space="PSUM") as ps_u,
        ):
            for hc in range(2):
                for qp in range(NQP):
                    u = [ps_u.tile([65, 512], F32, name=f"u{i}", tag=f"u{i}") for i in range(4)]
                    for kc in range(NSK):
                        # all 4 score MMs back-to-back: hi pairs land in
                        # different PE row groups and run concurrently
                        g = [
                            ps_s.tile([128, 1024], F32, tag=f"sg{hi}", name=f"sg{hi}")
                            for hi in range(2)
                        ]
                        for qh in range(2):
                            for hi in range(2):
                                hpart = slice(hi * 64, (hi + 1) * 64)
                                nc.tensor.matmul(
                                    g[hi][:, qh * 512:(qh + 1) * 512],
                                    lhsT=ke[hc][hpart, kc * 128:(kc + 1) * 128],
                                    rhs=qe[hc][hpart, (qp * 2 + qh) * 512:(qp * 2 + qh + 1) * 512],
                                    start=True,
                                    stop=True,
                                )
                        es = []
                        for hi in range(2):
                            e = e_pool.tile([128, 1024], MM_DT, name=f"e{hi}", tag=f"e{hi}")
                            nc.scalar.activation(
                                e, g[hi], mybir.ActivationFunctionType.Exp, scale=0.125
                            )
                            es.append(e)
                        for hi in range(2):
                            for qh in range(2):
                                nc.tensor.matmul(
                                    u[hi * 2 + qh],
                                    lhsT=vsb[:, kc, hc * 2 + hi, :],
                                    rhs=es[hi][:, qh * 512:(qh + 1) * 512],
                                    start=(kc == 0),
                                    stop=(kc == NSK - 1),
                                )
                    for hi in range(2):
                        for qh in range(2):
                            uu = u[hi * 2 + qh]
                            r = rcp_pool.tile([1, 512], F32)
                            nc.vector.reciprocal(r, uu[64:65, :])
                            rd = rdram_pool.tile([1, 512], F32)
                            nc.sync.dma_start(rd, r)
                            dbc_sb = rcp_pool.tile([64, 512], F32, tag="dbc_sb")
                            nc.sync.dma_start(dbc_sb, rd.partition_broadcast(64))
                            osl = slice((qp * 2 + qh) * 512, (qp * 2 + qh + 1) * 512)
                            nc.vector.tensor_mul(
                                ot64[:, hc * 2 + hi, osl], uu[0:64, :], dbc_sb
                            )

        if debug:
            nc.sync.dma_start(ap["ot_dbg"], ot64.bitcast(F32))

        # ====== Phase D: per-head scrambled output projection (full Wo) ======
        # out block rows mix 16 seq x 64 dim: row s2r contracts
        # O[s2r*16 + j, d] * Wo[j*64 + d, :].  rhs per j: strided view of ot64.
        ot_r = ot64.rearrange("p h (s2r g) -> p h s2r g", g=16)
        with (
            tc.tile_pool(name="wo_mc", bufs=2) as wo_pool,
            tc.tile_pool(name="ysb", bufs=3) as y_pool,
            tc.tile_pool(name="ps_y", bufs=3, space="PSUM") as ps_y,
        ):
            for mc in range(NKC):
                wo_mc = wo_pool.tile([64, 16, 128], MM_DT)
                nc.sync.dma_start(
                    wo_mc,
                    ap["wo"][:, mc * 128:(mc + 1) * 128]
                    .rearrange("(j p) m -> p j m", p=64)
                    .bitcast(MM_DT),
                )
                py = ps_y.tile([128, 512], F32)
                for j in range(16):
                    nc.tensor.matmul(
                        py,
                        lhsT=wo_mc[:, j, :],
                        rhs=ot_r[:, :, :, j],
                        start=(j == 0),
                        stop=(j == 15),
                    )
                ysb = y_pool.tile([128, 512], F32)
                nc.vector.tensor_copy(ysb, py)
                nc.sync.dma_start(ap["ypt"][mc * 128:(mc + 1) * 128, :], ysb)


def _build(debug=False):
    nc = bacc.Bacc("TRN2", target_bir_lowering=False, debug=False, num_devices=N_CORES)
    ap = {}
    ap["xt"] = nc.dram_tensor("xt", [DM, S], F32, kind="ExternalInput").ap()
    ap["wq"] = nc.dram_tensor("wq", [DM, GD], F32, kind="ExternalInput").ap()
    ap["wk"] = nc.dram_tensor("wk", [DM, GD], F32, kind="ExternalInput").ap()
    ap["wv"] = nc.dram_tensor("wv", [DM, GD], F32, kind="ExternalInput").ap()
    ap["wo"] = nc.dram_tensor("wo", [DM, DM], F32, kind="ExternalInput").ap()
    ap["bq2"] = nc.dram_tensor("bq2", [2, 128], F32, kind="ExternalInput").ap()
    ap["bk2"] = nc.dram_tensor("bk2", [2, 128], F32, kind="ExternalInput").ap()
    ap["bv"] = nc.dram_tensor("bv", [GD], F32, kind="ExternalInput").ap()
    ap["cosb"] = nc.dram_tensor("cosb", [128, S], F32, kind="ExternalInput").ap()
    ap["sinb"] = nc.dram_tensor("sinb", [128, S], F32, kind="ExternalInput").ap()
    ap["rot"] = nc.dram_tensor("rot", [128, 128], F32, kind="ExternalInput").ap()
    # per-core output: Y^T [1024, 512] (columns = 4 heads x 128 block rows)
    ap["ypt"] = nc.dram_tensor("ypt", [DM, 512], F32, kind="ExternalOutput").ap()
    if debug:
        ap["qe_dbg"] = nc.dram_tensor("qe_dbg", [2, 128, S], F32, kind="ExternalOutput").ap()
        ap["ke_dbg"] = nc.dram_tensor("ke_dbg", [2, 128, S], F32, kind="ExternalOutput").ap()
        ap["v_dbg"] = nc.dram_tensor("v_dbg", [128, NSK, 4, 65], F32, kind="ExternalOutput").ap()
        ap["ot_dbg"] = nc.dram_tensor("ot_dbg", [64, 4, S], F32, kind="ExternalOutput").ap()

    with tile.TileContext(nc) as tc:
        _emit(nc, tc, ap, debug=debug)
    nc.compile()
    return nc


_CACHE = {}


def _rope_tables():
    inv_freq = (1.0 / (10000.0 ** (np.arange(0, HD, 2, dtype=np.float32) / HD))).astype(np.float32)
    t = np.arange(S, dtype=np.float32)
    freqs = np.outer(t, inv_freq).astype(np.float32)  # [S, 32]
    emb = np.concatenate([freqs, freqs], axis=-1)  # [S, 64]
    cosT = np.cos(emb).astype(np.float32).T  # [64, S]
    sinT = np.sin(emb).astype(np.float32).T
    cosb = np.ascontiguousarray(np.concatenate([cosT, cosT], axis=0))  # [128, S]
    sinb = np.ascontiguousarray(np.concatenate([sinT, sinT], axis=0))
    return cosb, sinb


def _rot_matrix():
    p64 = np.zeros((HD, HD), dtype=np.float32)
    for i in range(32):
        p64[i, i + 32] = -1.0
        p64[i + 32, i] = 1.0
    p = np.zeros((128, 128), dtype=np.float32)
    p[0:64, 0:64] = p64
    p[64:128, 64:128] = p64
    return np.ascontiguousarray(p.T)  # lhsT = P^T


def kernel(x, Wq, bq, Wk, bk, Wv, bv, Wo, bo):
    x = np.asarray(x, dtype=np.float32)
    Wq, bq = np.asarray(Wq, np.float32), np.asarray(bq, np.float32)
    Wk, bk = np.asarray(Wk, np.float32), np.asarray(bk, np.float32)
    Wv, bv = np.asarray(Wv, np.float32), np.asarray(bv, np.float32)
    Wo, bo = np.asarray(Wo, np.float32), np.asarray(bo, np.float32)

    if "nc" not in _CACHE:
        _CACHE["nc"] = _build()
    nc = _CACHE["nc"]

    cosb, sinb = _rope_tables()
    rot = _rot_matrix()
    xt_b = [np.ascontiguousarray(x[b].T) for b in range(B)]  # [DM, S]
    wo_c = np.ascontiguousarray(Wo)

    in_maps = []
    for c in range(N_CORES):
        b, hg = divmod(c, HG)
        sl = slice(hg * GD, (hg + 1) * GD)
        in_maps.append(
            {
                "xt": xt_b[b],
                "wq": np.ascontiguousarray(Wq[:, sl]),
                "wk": np.ascontiguousarray(Wk[:, sl]),
                "wv": np.ascontiguousarray(Wv[:, sl]),
                "wo": wo_c,
                "bq2": np.ascontiguousarray(bq[sl].reshape(2, 128)),
                "bk2": np.ascontiguousarray(bk[sl].reshape(2, 128)),
                "bv": np.ascontiguousarray(bv[sl]),
                "cosb": cosb,
                "sinb": sinb,
                "rot": rot,
            }
        )

    res = bass_utils.run_bass_kernel_spmd(nc, in_maps, core_ids=list(range(N_CORES)))
    _CACHE["last_results"] = res

    # Block placement: core (b, hg), local head hl -> global head h = hg*4+hl,
    # lands at out[h//8, (h%8)*256 + b*128 : +128, :].
    out = np.empty((B, S, DM), dtype=np.float32)
    for c in range(N_CORES):
        b, hg = divmod(c, HG)
        ypt = res.results[c]["ypt"]  # [1024, 512]
        for hl in range(4):
            h = hg * 4 + hl
            b2 = h // 8
            s2 = (h % 8) * 256 + b * 128
            out[b2, s2:s2 + 128, :] = ypt[:, hl * 128:(hl + 1) * 128].T
    out += bo[None, None, :]
    return out


# revision 26
# speedup vs baseline: 1.6172x; 1.2700x over previous
"""Multi-head attention (RoPE) Trainium2 Bass kernel.

Problem: B=2, S=2048, d_model=1024, 16 heads x head_dim 64, fp32.

The reference faithfully replicates a torch rank-5 reshape bug: the
attention output [1,H,B,S,D] is transposed to [1,H,B,S,D]->(0,2,1,3,4)
and flat-reshaped to [B,S,H*D] BEFORE the Wo projection. Net semantics:
  out[b2, s2, :] = flatten(O[b, h, s0:s0+16, :]) @ Wo + bo
  with h = b2*8 + s2//256, b = (s2//128)%2, s0 = (s2%128)*16,
so the projection is PER-HEAD (contraction mixes 16 seq x 64 dims of one
head) and every (b,h) yields an independent [128, 1024] output block.

Sharding (8 cores): batch (2) x head groups (4 groups of 4 heads).
Per core: QKV slices via f32r matmuls in transposed layout, RoPE
(rotate-half via a signed permutation matmul), per-head attention with
unnormalized softmax (ones-column appended to V gives the denominator),
normalize into ot64 [64, 4head, S], then per-head scrambled projection
against full Wo. Host places the 32 independent blocks and adds bo.
"""

import numpy as np

import concourse.bass as bass
import concourse.tile as tile
from concourse import bacc, mybir
from concourse import bass_utils

F32 = mybir.dt.float32
MM_DT = mybir.dt.float32r  # matmul operand dtype (float32r: 1 cyc/row)
USE_DUP = False  # duplicate ot into partitions 64-127 for phase D row packing

B, S, DM, H, HD = 2, 2048, 1024, 16, 64
N_CORES = 8
HG = 4          # head groups (tensor-parallel factor)
GD = DM // HG   # qkv dims per core = 256
NKC = DM // 128   # d_model contraction chunks = 8
NST = S // 512    # seq tiles of 512 = 4
NSK = S // 128    # seq_k chunks of 128 = 16
NQP = S // 1024   # seq_q pairs of 1024 = 2


def _emit(nc, tc, ap, debug=False):
    import contextlib

    ctx = contextlib.ExitStack()
    with ctx:
        consts = ctx.enter_context(tc.tile_pool(name="consts", bufs=1))
        big = ctx.enter_context(tc.tile_pool(name="big", bufs=1))

        # ---- constants / weights to SBUF ----
        cosb = consts.tile([128, S], F32)
        nc.sync.dma_start(cosb, ap["cosb"])
        sinb = consts.tile([128, S], F32)
        nc.sync.dma_start(sinb, ap["sinb"])
        rot = consts.tile([128, 128], MM_DT)
        nc.sync.dma_start(rot, ap["rot"].bitcast(MM_DT))
        bqc = consts.tile([128, 2], F32)
        nc.gpsimd.dma_start(bqc, ap["bq2"].rearrange("c p -> p c"))
        bkc = consts.tile([128, 2], F32)
        nc.gpsimd.dma_start(bkc, ap["bk2"].rearrange("c p -> p c"))
        bvb = consts.tile([128, GD], F32)
        nc.gpsimd.dma_start(bvb, ap["bv"].partition_broadcast(128))

        wq = consts.tile([128, NKC, GD], MM_DT)
        nc.sync.dma_start(wq, ap["wq"].rearrange("(kc p) m -> p kc m", p=128).bitcast(MM_DT))
        wk = consts.tile([128, NKC, GD], MM_DT)
        nc.sync.dma_start(wk, ap["wk"].rearrange("(kc p) m -> p kc m", p=128).bitcast(MM_DT))
        wv = consts.tile([128, NKC, GD], MM_DT)
        nc.sync.dma_start(wv, ap["wv"].rearrange("(kc p) m -> p kc m", p=128).bitcast(MM_DT))

        # ---- persistent activation buffers ----
        qe = [big.tile([128, S], MM_DT, name=f"qe{mc}", tag=f"qe{mc}") for mc in range(2)]
        ke = [big.tile([128, S], MM_DT, name=f"ke{mc}", tag=f"ke{mc}") for mc in range(2)]
        # V natural layout + ones column: [128 seq, kc, head, 65]
        vsb = big.tile([128, NSK, 4, 65], MM_DT, name="vsb", tag="vsb")
        nc.vector.memset(vsb[:, :, :, 64:65].bitcast(F32), 1.0)
        # normalized attention output, heads on the free axis: [128, head, S].
        # Partitions 0-63 hold O^T; 64-127 hold a DMA-duplicated copy so the
        # Wo matmuls can alternate row groups (odd j chunks at base 64).
        ot64 = big.tile([128, 4, S], MM_DT, name="ot64", tag="ot64")

        # ================= Phase B: QKV projections + RoPE =================
        with (
            tc.tile_pool(name="xt", bufs=6) as xt_pool,
            tc.tile_pool(name="raw", bufs=3) as raw_pool,
            tc.tile_pool(name="t1", bufs=3) as t1_pool,
            tc.tile_pool(name="ps_qk", bufs=1, space="PSUM") as ps_qk,
            tc.tile_pool(name="ps_v", bufs=1, space="PSUM") as ps_v,
        ):
            for st in range(NST):
                sl = slice(st * 512, (st + 1) * 512)
                pqk = {}
                pv = {}
                for tgt in range(2):
                    for mc in range(2):
                        pqk[tgt, mc] = ps_qk.tile(
                            [128, 512], F32, name=f"pqk{tgt}{mc}", tag=f"qk{tgt}{mc}"
                        )
                for ss in range(4):
                    pv[ss] = ps_v.tile([128, GD], F32, name=f"pv{ss}", tag=f"v{ss}")
                for kc in range(NKC):
                    xt_kc = xt_pool.tile([128, 512], MM_DT)
                    nc.sync.dma_start(
                        xt_kc,
                        ap["xt"][kc * 128:(kc + 1) * 128, sl].bitcast(MM_DT),
                    )
                    for tgt in range(2):
                        w_sb = wq if tgt == 0 else wk
                        for mc in range(2):
                            nc.tensor.matmul(
                                pqk[tgt, mc],
                                lhsT=w_sb[:, kc, mc * 128:(mc + 1) * 128],
                                rhs=xt_kc,
                                start=(kc == 0),
                                stop=(kc == NKC - 1),
                            )
                    for ss in range(4):
                        nc.tensor.matmul(
                            pv[ss],
                            lhsT=xt_kc[:, ss * 128:(ss + 1) * 128],
                            rhs=wv[:, kc, :],
                            start=(kc == 0),
                            stop=(kc == NKC - 1),
                        )
                for tgt in range(2):
                    bias = bqc if tgt == 0 else bkc
                    dst = qe if tgt == 0 else ke
                    for mc in range(2):
                        raw = raw_pool.tile([128, 512], MM_DT)
                        nc.vector.tensor_scalar_add(raw, pqk[tgt, mc], bias[:, mc:mc + 1])
                        rps = ps_qk.tile(
                            [128, 512], F32, name=f"rps{tgt}{mc}", tag=f"qk{tgt}{mc}"
                        )
                        nc.tensor.matmul(rps, lhsT=rot, rhs=raw, start=True, stop=True)
                        t1 = t1_pool.tile([128, 512], F32)
                        nc.vector.tensor_mul(t1, rps, sinb[:, sl])
                        d = dst[mc][:, sl]
                        nc.vector.tensor_mul(d, raw, cosb[:, sl])
                        nc.vector.tensor_add(d, d, t1)
                for ss in range(4):
                    nc.vector.tensor_add(
                        vsb[:, st * 4 + ss, :, 0:64],
                        pv[ss].rearrange("p (h d) -> p h d", h=4),
                        bvb.rearrange("p (h d) -> p h d", h=4),
                    )

        if debug:
            for mc in range(2):
                nc.sync.dma_start(ap["qe_dbg"][mc], qe[mc].bitcast(F32))
                nc.sync.dma_start(ap["ke_dbg"][mc], ke[mc].bitcast(F32))
            nc.sync.dma_start(ap["v_dbg"], vsb.bitcast(F32))

        # ================= Phase C: attention =================
        LAG = 2  # AV matmuls trail score matmuls by LAG kc iterations
        with (
            tc.tile_pool(name="e", bufs=LAG + 2) as e_pool,
            tc.tile_pool(name="rcp", bufs=4) as rcp_pool,
            tc.tile_pool(name="usb", bufs=2) as usb_pool,
            tc.tile_pool(name="rdram", bufs=4, space="DRAM") as rdram_pool,
            tc.tile_pool(name="ps_s", bufs=3, space="PSUM") as ps_s,
            tc.tile_pool(name="ps_u", bufs=1, space="PSUM") as ps_u,
        ):
            for hc in range(2):
                for qt in range(NST):
                    qsl = slice(qt * 512, (qt + 1) * 512)
                    u = [ps_u.tile([65, 512], F32, name=f"u{i}", tag=f"u{i}") for i in range(2)]
                    es = {}
                    for kc in range(NSK + LAG):
                        if kc >= LAG:
                            ka = kc - LAG
                            for hi in range(2):
                                nc.tensor.matmul(
                                    u[hi],
                                    lhsT=vsb[:, ka, hc * 2 + hi, :],
                                    rhs=es[ka][:, hi * 512:(hi + 1) * 512],
                                    start=(ka == 0),
                                    stop=(ka == NSK - 1),
                                )
                            if ka > 0:
                                del es[ka - 1]
                        if kc < NSK:
                            # both heads' scores side by side in one 2-bank
                            # group; hi pairs use different PE row groups and
                            # run concurrently
                            g = ps_s.tile([128, 1024], F32, tag="sg", name="sg")
                            for hi in range(2):
                                hpart = slice(hi * 64, (hi + 1) * 64)
                                nc.tensor.matmul(
                                    g[:, hi * 512:(hi + 1) * 512],
                                    lhsT=ke[hc][hpart, kc * 128:(kc + 1) * 128],
                                    rhs=qe[hc][hpart, qsl],
                                    start=True,
                                    stop=True,
                                )
                            e = e_pool.tile([128, 1024], MM_DT, name="e", tag="e")
                            nc.scalar.activation(
                                e, g, mybir.ActivationFunctionType.Exp, scale=0.125
                            )
                            es[kc] = e
                    for hi in range(2):
                        # copy U off PSUM immediately so the bank frees early
                        usb = usb_pool.tile([65, 512], F32)
                        nc.vector.tensor_copy(usb, u[hi])
                        r = rcp_pool.tile([1, 512], F32)
                        nc.vector.reciprocal(r, usb[64:65, :])
                        rd = rdram_pool.tile([1, 512], F32)
                        nc.sync.dma_start(rd, r)
                        dbc_sb = rcp_pool.tile([64, 512], F32, tag="dbc_sb")
                        nc.sync.dma_start(dbc_sb, rd.partition_broadcast(64))
                        nc.vector.tensor_mul(
                            ot64[0:64, hc * 2 + hi, qsl], usb[0:64, :], dbc_sb
                        )
                        if USE_DUP:
                            # duplicate into partitions 64-127 for phase D row packing
                            nc.sync.dma_start(
                                ot64[64:128, hc * 2 + hi, qsl], ot64[0:64, hc * 2 + hi, qsl]
                            )

        if debug:
            nc.sync.dma_start(ap["ot_dbg"], ot64[0:64].bitcast(F32))

        # ====== Phase D: per-head scrambled output projection (full Wo) ======
        # out block rows mix 16 seq x 64 dim: row s2r contracts
        # O[s2r*16 + j, d] * Wo[j*64 + d, :].  rhs per j: strided view of ot64.
        ot_r = ot64.rearrange("p h (s2r g) -> p h s2r g", g=16)
        with (
            tc.tile_pool(name="wo_mc", bufs=3) as wo_pool,
            tc.tile_pool(name="ysb", bufs=3) as y_pool,
            tc.tile_pool(name="ps_y", bufs=3, space="PSUM") as ps_y,
        ):
            for mc in range(NKC):
                # row r of Wo maps to (partition r%128, chunk r//128); chunk j
                # of the contraction (j = seq offset in the 16-group) reads
                # partitions (j%2)*64..+64 of chunk j//2 — even j at base 0,
                # odd j at base 64 (row-group concurrency with the dup copy).
                if USE_DUP:
                    wo_mc = wo_pool.tile([128, 8, 128], MM_DT, name="wo_mc", tag="wo_mc")
                    nc.sync.dma_start(
                        wo_mc,
                        ap["wo"][:, mc * 128:(mc + 1) * 128]
                        .rearrange("(c p) m -> p c m", p=128)
                        .bitcast(MM_DT),
                    )
                else:
                    wo_mc = wo_pool.tile([64, 16, 128], MM_DT, name="wo_mc", tag="wo_mc")
                    nc.sync.dma_start(
                        wo_mc,
                        ap["wo"][:, mc * 128:(mc + 1) * 128]
                        .rearrange("(j p) m -> p j m", p=64)
                        .bitcast(MM_DT),
                    )
                py = ps_y.tile([128, 512], F32)
                for j in range(16):
                    if USE_DUP:
                        base = slice((j % 2) * 64, (j % 2) * 64 + 64)
                        lhsT = wo_mc[base, j // 2, :]
                        rhs = ot_r[base, :, :, j]
                    else:
                        lhsT = wo_mc[:, j, :]
                        rhs = ot_r[0:64, :, :, j]
                    nc.tensor.matmul(
                        py,
                        lhsT=lhsT,
                        rhs=rhs,
                        start=(j == 0),
                        stop=(j == 15),
                    )
                ysb = y_pool.tile([128, 512], F32)
                nc.vector.tensor_copy(ysb, py)
                nc.sync.dma_start(ap["ypt"][mc * 128:(mc + 1) * 128, :], ysb)


def _build(debug=False):
    nc = bacc.Bacc("TRN2", target_bir_lowering=False, debug=False, num_devices=N_CORES)
    ap = {}
    ap["xt"] = nc.dram_tensor("xt", [DM, S], F32, kind="ExternalInput").ap()
    ap["wq"] = nc.dram_tensor("wq", [DM, GD], F32, kind="ExternalInput").ap()
    ap["wk"] = nc.dram_tensor("wk", [DM, GD], F32, kind="ExternalInput").ap()
    ap["wv"] = nc.dram_tensor("wv", [DM, GD], F32, kind="ExternalInput").ap()
    ap["wo"] = nc.dram_tensor("wo", [DM, DM], F32, kind="ExternalInput").ap()
    ap["bq2"] = nc.dram_tensor("bq2", [2, 128], F32, kind="ExternalInput").ap()
    ap["bk2"] = nc.dram_tensor("bk2", [2, 128], F32, kind="ExternalInput").ap()
    ap["bv"] = nc.dram_tensor("bv", [GD], F32, kind="ExternalInput").ap()
    ap["cosb"] = nc.dram_tensor("cosb", [128, S], F32, kind="ExternalInput").ap()
    ap["sinb"] = nc.dram_tensor("sinb", [128, S], F32, kind="ExternalInput").ap()
    ap["rot"] = nc.dram_tensor("rot", [128, 128], F32, kind="ExternalInput").ap()
    # per-core output: Y^T [1024, 512] (columns = 4 heads x 128 block rows)
    ap["ypt"] = nc.dram_tensor("ypt", [DM, 512], F32, kind="ExternalOutput").ap()
    if debug:
        ap["qe_dbg"] = nc.dram_tensor("qe_dbg", [2, 128, S], F32, kind="ExternalOutput").ap()
        ap["ke_dbg"] = nc.dram_tensor("ke_dbg", [2, 128, S], F32, kind="ExternalOutput").ap()
        ap["v_dbg"] = nc.dram_tensor("v_dbg", [128, NSK, 4, 65], F32, kind="ExternalOutput").ap()
        ap["ot_dbg"] = nc.dram_tensor("ot_dbg", [64, 4, S], F32, kind="ExternalOutput").ap()

    with tile.TileContext(nc) as tc:
        _emit(nc, tc, ap, debug=debug)
    nc.compile()
    return nc


_CACHE = {}


def _rope_tables():
    inv_freq = (1.0 / (10000.0 ** (np.arange(0, HD, 2, dtype=np.float32) / HD))).astype(np.float32)
    t = np.arange(S, dtype=np.float32)
    freqs = np.outer(t, inv_freq).astype(np.float32)  # [S, 32]
    emb = np.concatenate([freqs, freqs], axis=-1)  # [S, 64]
    cosT = np.cos(emb).astype(np.float32).T  # [64, S]
    sinT = np.sin(emb).astype(np.float32).T
    cosb = np.ascontiguousarray(np.concatenate([cosT, cosT], axis=0))  # [128, S]
    sinb = np.ascontiguousarray(np.concatenate([sinT, sinT], axis=0))
    return cosb, sinb


def _rot_matrix():
    p64 = np.zeros((HD, HD), dtype=np.float32)
    for i in range(32):
        p64[i, i + 32] = -1.0
        p64[i + 32, i] = 1.0
    p = np.zeros((128, 128), dtype=np.float32)
    p[0:64, 0:64] = p64
    p[64:128, 64:128] = p64
    return np.ascontiguousarray(p.T)  # lhsT = P^T


def kernel(x, Wq, bq, Wk, bk, Wv, bv, Wo, bo):
    x = np.asarray(x, dtype=np.float32)
    Wq, bq = np.asarray(Wq, np.float32), np.asarray(bq, np.float32)
    Wk, bk = np.asarray(Wk, np.float32), np.asarray(bk, np.float32)
    Wv, bv = np.asarray(Wv, np.float32), np.asarray(bv, np.float32)
    Wo, bo = np.asarray(Wo, np.float32), np.asarray(bo, np.float32)

    if "nc" not in _CACHE:
        _CACHE["nc"] = _build()
    nc = _CACHE["nc"]

    cosb, sinb = _rope_tables()
    rot = _rot_matrix()
    xt_b = [np.ascontiguousarray(x[b].T) for b in range(B)]  # [DM, S]
    wo_c = np.ascontiguousarray(Wo)

    in_maps = []
    for c in range(N_CORES):
        b, hg = divmod(c, HG)
        sl = slice(hg * GD, (hg + 1) * GD)
        in_maps.append(
            {
                "xt": xt_b[b],
                "wq": np.ascontiguousarray(Wq[:, sl]),
                "wk": np.ascontiguousarray(Wk[:, sl]),
                "wv": np.ascontiguousarray(Wv[:, sl]),
                "wo": wo_c,
                "bq2": np.ascontiguousarray(bq[sl].reshape(2, 128)),
                "bk2": np.ascontiguousarray(bk[sl].reshape(2, 128)),
                "bv": np.ascontiguousarray(bv[sl]),
                "cosb": cosb,
                "sinb": sinb,
                "rot": rot,
            }
        )

    res = bass_utils.run_bass_kernel_spmd(nc, in_maps, core_ids=list(range(N_CORES)))
    _CACHE["last_results"] = res

    # Block placement: core (b, hg), local head hl -> global head h = hg*4+hl,
    # lands at out[h//8, (h%8)*256 + b*128 : +128, :].
    out = np.empty((B, S, DM), dtype=np.float32)
    for c in range(N_CORES):
        b, hg = divmod(c, HG)
        ypt = res.results[c]["ypt"]  # [1024, 512]
        for hl in range(4):
            h = hg * 4 + hl
            b2 = h // 8
            s2 = (h % 8) * 256 + b * 128
            out[b2, s2:s2 + 128, :] = ypt[:, hl * 128:(hl + 1) * 128].T
    out += bo[None, None, :]
    return out


# revision 29
# speedup vs baseline: 1.6648x; 1.0294x over previous
"""Multi-head attention (RoPE) Trainium2 Bass kernel.

Problem: B=2, S=2048, d_model=1024, 16 heads x head_dim 64, fp32.

The reference faithfully replicates a torch rank-5 reshape bug: the
attention output [1,H,B,S,D] is transposed to [1,H,B,S,D]->(0,2,1,3,4)
and flat-reshaped to [B,S,H*D] BEFORE the Wo projection. Net semantics:
  out[b2, s2, :] = flatten(O[b, h, s0:s0+16, :]) @ Wo + bo
  with h = b2*8 + s2//256, b = (s2//128)%2, s0 = (s2%128)*16,
so the projection is PER-HEAD (contraction mixes 16 seq x 64 dims of one
head) and every (b,h) yields an independent [128, 1024] output block.

Sharding (8 cores): batch (2) x head groups (4 groups of 4 heads).
Per core: QKV slices via f32r matmuls in transposed layout, RoPE
(rotate-half via a signed permutation matmul), per-head attention with
unnormalized softmax (ones-column appended to V gives the denominator),
normalize into ot64 [64, 4head, S], then per-head scrambled projection
against full Wo. Host places the 32 independent blocks and adds bo.
"""

import numpy as np

import concourse.bass as bass
import concourse.tile as tile
from concourse import bacc, mybir
from concourse import bass_utils

F32 = mybir.dt.float32
MM_DT = mybir.dt.float32r  # matmul operand dtype (float32r: 1 cyc/row)
USE_DUP = True  # duplicate ot into partitions 64-127 for phase D row packing

B, S, DM, H, HD = 2, 2048, 1024, 16, 64
N_CORES = 8
HG = 4          # head groups (tensor-parallel factor)
GD = DM // HG   # qkv dims per core = 256
NKC = DM // 128   # d_model contraction chunks = 8
NST = S // 512    # seq tiles of 512 = 4
NSK = S // 128    # seq_k chunks of 128 = 16
NQP = S // 1024   # seq_q pairs of 1024 = 2


def _emit(nc, tc, ap, debug=False):
    import contextlib

    ctx = contextlib.ExitStack()
    with ctx:
        consts = ctx.enter_context(tc.tile_pool(name="consts", bufs=1))
        big = ctx.enter_context(tc.tile_pool(name="big", bufs=1))

        # ---- constants / weights to SBUF ----
        cosb = consts.tile([128, S], F32)
        nc.sync.dma_start(cosb, ap["cosb"])
        sinb = consts.tile([128, S], F32)
        nc.sync.dma_start(sinb, ap["sinb"])
        rot = consts.tile([128, 128], MM_DT)
        nc.sync.dma_start(rot, ap["rot"].bitcast(MM_DT))
        bqc = consts.tile([128, 2], F32)
        nc.gpsimd.dma_start(bqc, ap["bq2"].rearrange("c p -> p c"))
        bkc = consts.tile([128, 2], F32)
        nc.gpsimd.dma_start(bkc, ap["bk2"].rearrange("c p -> p c"))
        bvb = consts.tile([128, GD], F32)
        nc.gpsimd.dma_start(bvb, ap["bv"].partition_broadcast(128))

        wq = consts.tile([128, NKC, GD], MM_DT)
        nc.sync.dma_start(wq, ap["wq"].rearrange("(kc p) m -> p kc m", p=128).bitcast(MM_DT))
        wk = consts.tile([128, NKC, GD], MM_DT)
        nc.sync.dma_start(wk, ap["wk"].rearrange("(kc p) m -> p kc m", p=128).bitcast(MM_DT))
        wv = consts.tile([128, NKC, GD], MM_DT)
        nc.sync.dma_start(wv, ap["wv"].rearrange("(kc p) m -> p kc m", p=128).bitcast(MM_DT))

        # ---- persistent activation buffers ----
        qe = [big.tile([128, S], MM_DT, name=f"qe{mc}", tag=f"qe{mc}") for mc in range(2)]
        ke = [big.tile([128, S], MM_DT, name=f"ke{mc}", tag=f"ke{mc}") for mc in range(2)]
        # V natural layout + ones column: [128 seq, kc, head, 65]
        vsb = big.tile([128, NSK, 4, 65], MM_DT, name="vsb", tag="vsb")
        nc.vector.memset(vsb[:, :, :, 64:65].bitcast(F32), 1.0)
        # normalized attention output, heads on the free axis: [128, head, S].
        # Partitions 0-63 hold O^T; 64-127 hold a DMA-duplicated copy so the
        # Wo matmuls can alternate row groups (odd j chunks at base 64).
        ot64 = big.tile([128, 4, S], MM_DT, name="ot64", tag="ot64")

        # ================= Phase B: QKV projections + RoPE =================
        with (
            tc.tile_pool(name="xt", bufs=6) as xt_pool,
            tc.tile_pool(name="raw", bufs=3) as raw_pool,
            tc.tile_pool(name="t1", bufs=3) as t1_pool,
            tc.tile_pool(name="ps_qk", bufs=1, space="PSUM") as ps_qk,
            tc.tile_pool(name="ps_v", bufs=1, space="PSUM") as ps_v,
        ):
            for st in range(NST):
                sl = slice(st * 512, (st + 1) * 512)
                pqk = {}
                pv = {}
                for tgt in range(2):
                    for mc in range(2):
                        pqk[tgt, mc] = ps_qk.tile(
                            [128, 512], F32, name=f"pqk{tgt}{mc}", tag=f"qk{tgt}{mc}"
                        )
                for ss in range(4):
                    pv[ss] = ps_v.tile([128, GD], F32, name=f"pv{ss}", tag=f"v{ss}")
                for kc in range(NKC):
                    xt_kc = xt_pool.tile([128, 512], MM_DT)
                    nc.sync.dma_start(
                        xt_kc,
                        ap["xt"][kc * 128:(kc + 1) * 128, sl].bitcast(MM_DT),
                    )
                    for tgt in range(2):
                        w_sb = wq if tgt == 0 else wk
                        for mc in range(2):
                            nc.tensor.matmul(
                                pqk[tgt, mc],
                                lhsT=w_sb[:, kc, mc * 128:(mc + 1) * 128],
                                rhs=xt_kc,
                                start=(kc == 0),
                                stop=(kc == NKC - 1),
                            )
                    for ss in range(4):
                        nc.tensor.matmul(
                            pv[ss],
                            lhsT=xt_kc[:, ss * 128:(ss + 1) * 128],
                            rhs=wv[:, kc, :],
                            start=(kc == 0),
                            stop=(kc == NKC - 1),
                        )
                for tgt in range(2):
                    bias = bqc if tgt == 0 else bkc
                    dst = qe if tgt == 0 else ke
                    for mc in range(2):
                        raw = raw_pool.tile([128, 512], MM_DT)
                        nc.vector.tensor_scalar_add(raw, pqk[tgt, mc], bias[:, mc:mc + 1])
                        rps = ps_qk.tile(
                            [128, 512], F32, name=f"rps{tgt}{mc}", tag=f"qk{tgt}{mc}"
                        )
                        nc.tensor.matmul(rps, lhsT=rot, rhs=raw, start=True, stop=True)
                        t1 = t1_pool.tile([128, 512], F32)
                        nc.vector.tensor_mul(t1, rps, sinb[:, sl])
                        d = dst[mc][:, sl]
                        nc.vector.tensor_mul(d, raw, cosb[:, sl])
                        nc.vector.tensor_add(d, d, t1)
                for ss in range(4):
                    nc.vector.tensor_add(
                        vsb[:, st * 4 + ss, :, 0:64],
                        pv[ss].rearrange("p (h d) -> p h d", h=4),
                        bvb.rearrange("p (h d) -> p h d", h=4),
                    )

        if debug:
            for mc in range(2):
                nc.sync.dma_start(ap["qe_dbg"][mc], qe[mc].bitcast(F32))
                nc.sync.dma_start(ap["ke_dbg"][mc], ke[mc].bitcast(F32))
            nc.sync.dma_start(ap["v_dbg"], vsb.bitcast(F32))

        # ================= Phase C: attention =================
        LAG = 2  # AV matmuls trail score matmuls by LAG kc iterations
        with (
            tc.tile_pool(name="e", bufs=LAG + 2) as e_pool,
            tc.tile_pool(name="rcp", bufs=4) as rcp_pool,
            tc.tile_pool(name="usb", bufs=2) as usb_pool,
            tc.tile_pool(name="rdram", bufs=4, space="DRAM") as rdram_pool,
            tc.tile_pool(name="ps_s", bufs=3, space="PSUM") as ps_s,
            tc.tile_pool(name="ps_u", bufs=1, space="PSUM") as ps_u,
        ):
            for hc in range(2):
                for qt in range(NST):
                    qsl = slice(qt * 512, (qt + 1) * 512)
                    u = [ps_u.tile([65, 512], F32, name=f"u{i}", tag=f"u{i}") for i in range(2)]
                    es = {}
                    for kc in range(NSK + LAG):
                        if kc >= LAG:
                            ka = kc - LAG
                            for hi in range(2):
                                nc.tensor.matmul(
                                    u[hi],
                                    lhsT=vsb[:, ka, hc * 2 + hi, :],
                                    rhs=es[ka][:, hi * 512:(hi + 1) * 512],
                                    start=(ka == 0),
                                    stop=(ka == NSK - 1),
                                )
                            if ka > 0:
                                del es[ka - 1]
                        if kc < NSK:
                            # both heads' scores side by side in one 2-bank
                            # group; hi pairs use different PE row groups and
                            # run concurrently
                            g = ps_s.tile([128, 1024], F32, tag="sg", name="sg")
                            for hi in range(2):
                                hpart = slice(hi * 64, (hi + 1) * 64)
                                nc.tensor.matmul(
                                    g[:, hi * 512:(hi + 1) * 512],
                                    lhsT=ke[hc][hpart, kc * 128:(kc + 1) * 128],
                                    rhs=qe[hc][hpart, qsl],
                                    start=True,
                                    stop=True,
                                )
                            e = e_pool.tile([128, 1024], MM_DT, name="e", tag="e")
                            nc.scalar.activation(
                                e, g, mybir.ActivationFunctionType.Exp, scale=0.125
                            )
                            es[kc] = e
                    for hi in range(2):
                        # copy U off PSUM immediately so the bank frees early
                        usb = usb_pool.tile([65, 512], F32)
                        nc.vector.tensor_copy(usb, u[hi])
                        r = rcp_pool.tile([1, 512], F32)
                        nc.vector.reciprocal(r, usb[64:65, :])
                        rd = rdram_pool.tile([1, 512], F32)
                        nc.sync.dma_start(rd, r)
                        dbc_sb = rcp_pool.tile([64, 512], F32, tag="dbc_sb")
                        nc.sync.dma_start(dbc_sb, rd.partition_broadcast(64))
                        nc.vector.tensor_mul(
                            ot64[0:64, hc * 2 + hi, qsl], usb[0:64, :], dbc_sb
                        )
                        if USE_DUP:
                            # duplicate into partitions 64-127 for phase D row packing
                            nc.gpsimd.dma_start(
                                ot64[64:128, hc * 2 + hi, qsl], ot64[0:64, hc * 2 + hi, qsl]
                            )

        if debug:
            nc.sync.dma_start(ap["ot_dbg"], ot64[0:64].bitcast(F32))

        # ====== Phase D: per-head scrambled output projection (full Wo) ======
        # out block rows mix 16 seq x 64 dim: row s2r contracts
        # O[s2r*16 + j, d] * Wo[j*64 + d, :].  rhs per j: strided view of ot64.
        ot_r = ot64.rearrange("p h (s2r g) -> p h s2r g", g=16)
        with (
            tc.tile_pool(name="wo_mc", bufs=3) as wo_pool,
            tc.tile_pool(name="ysb", bufs=3) as y_pool,
            tc.tile_pool(name="ps_y", bufs=3, space="PSUM") as ps_y,
        ):
            for mc in range(NKC):
                # row r of Wo maps to (partition r%128, chunk r//128); chunk j
                # of the contraction (j = seq offset in the 16-group) reads
                # partitions (j%2)*64..+64 of chunk j//2 — even j at base 0,
                # odd j at base 64 (row-group concurrency with the dup copy).
                if USE_DUP:
                    wo_mc = wo_pool.tile([128, 8, 128], MM_DT, name="wo_mc", tag="wo_mc")
                    nc.sync.dma_start(
                        wo_mc,
                        ap["wo"][:, mc * 128:(mc + 1) * 128]
                        .rearrange("(c p) m -> p c m", p=128)
                        .bitcast(MM_DT),
                    )
                else:
                    wo_mc = wo_pool.tile([64, 16, 128], MM_DT, name="wo_mc", tag="wo_mc")
                    nc.sync.dma_start(
                        wo_mc,
                        ap["wo"][:, mc * 128:(mc + 1) * 128]
                        .rearrange("(j p) m -> p j m", p=64)
                        .bitcast(MM_DT),
                    )
                if USE_DUP:
                    # even/odd j chunks alternate PE row groups (0-63 / 64-127)
                    # into two separate accumulators -> concurrent pairs.
                    py_a = ps_y.tile([128, 512], F32, name="py_a", tag="py_a")
                    py_b = ps_y.tile([128, 512], F32, name="py_b", tag="py_b")
                    for j in range(16):
                        base = slice((j % 2) * 64, (j % 2) * 64 + 64)
                        nc.tensor.matmul(
                            py_a if j % 2 == 0 else py_b,
                            lhsT=wo_mc[base, j // 2, :],
                            rhs=ot_r[base, :, :, j],
                            start=(j < 2),
                            stop=(j >= 14),
                        )
                    ta = y_pool.tile([128, 512], F32, name="ta", tag="ta")
                    nc.vector.tensor_copy(ta, py_a)
                    ysb = y_pool.tile([128, 512], F32, name="ysb", tag="ysb")
                    nc.vector.tensor_add(ysb, py_b, ta)
                else:
                    py = ps_y.tile([128, 512], F32)
                    for j in range(16):
                        nc.tensor.matmul(
                            py,
                            lhsT=wo_mc[:, j, :],
                            rhs=ot_r[0:64, :, :, j],
                            start=(j == 0),
                            stop=(j == 15),
                        )
                    ysb = y_pool.tile([128, 512], F32, name="ysb", tag="ysb")
                    nc.vector.tensor_copy(ysb, py)
                nc.sync.dma_start(ap["ypt"][mc * 128:(mc + 1) * 128, :], ysb)


def _build(debug=False):
    nc = bacc.Bacc("TRN2", target_bir_lowering=False, debug=False, num_devices=N_CORES)
    ap = {}
    ap["xt"] = nc.dram_tensor("xt", [DM, S], F32, kind="ExternalInput").ap()
    ap["wq"] = nc.dram_tensor("wq", [DM, GD], F32, kind="ExternalInput").ap()
    ap["wk"] = nc.dram_tensor("wk", [DM, GD], F32, kind="ExternalInput").ap()
    ap["wv"] = nc.dram_tensor("wv", [DM, GD], F32, kind="ExternalInput").ap()
    ap["wo"] = nc.dram_tensor("wo", [DM, DM], F32, kind="ExternalInput").ap()
    ap["bq2"] = nc.dram_tensor("bq2", [2, 128], F32, kind="ExternalInput").ap()
    ap["bk2"] = nc.dram_tensor("bk2", [2, 128], F32, kind="ExternalInput").ap()
    ap["bv"] = nc.dram_tensor("bv", [GD], F32, kind="ExternalInput").ap()
    ap["cosb"] = nc.dram_tensor("cosb", [128, S], F32, kind="ExternalInput").ap()
    ap["sinb"] = nc.dram_tensor("sinb", [128, S], F32, kind="ExternalInput").ap()
    ap["rot"] = nc.dram_tensor("rot", [128, 128], F32, kind="ExternalInput").ap()
    # per-core output: Y^T [1024, 512] (columns = 4 heads x 128 block rows)
    ap["ypt"] = nc.dram_tensor("ypt", [DM, 512], F32, kind="ExternalOutput").ap()
    if debug:
        ap["qe_dbg"] = nc.dram_tensor("qe_dbg", [2, 128, S], F32, kind="ExternalOutput").ap()
        ap["ke_dbg"] = nc.dram_tensor("ke_dbg", [2, 128, S], F32, kind="ExternalOutput").ap()
        ap["v_dbg"] = nc.dram_tensor("v_dbg", [128, NSK, 4, 65], F32, kind="ExternalOutput").ap()
        ap["ot_dbg"] = nc.dram_tensor("ot_dbg", [64, 4, S], F32, kind="ExternalOutput").ap()

    with tile.TileContext(nc) as tc:
        _emit(nc, tc, ap, debug=debug)
    nc.compile()
    return nc


_CACHE = {}


def _rope_tables():
    inv_freq = (1.0 / (10000.0 ** (np.arange(0, HD, 2, dtype=np.float32) / HD))).astype(np.float32)
    t = np.arange(S, dtype=np.float32)
    freqs = np.outer(t, inv_freq).astype(np.float32)  # [S, 32]
    emb = np.concatenate([freqs, freqs], axis=-1)  # [S, 64]
    cosT = np.cos(emb).astype(np.float32).T  # [64, S]
    sinT = np.sin(emb).astype(np.float32).T
    cosb = np.ascontiguousarray(np.concatenate([cosT, cosT], axis=0))  # [128, S]
    sinb = np.ascontiguousarray(np.concatenate([sinT, sinT], axis=0))
    return cosb, sinb


def _rot_matrix():
    p64 = np.zeros((HD, HD), dtype=np.float32)
    for i in range(32):
        p64[i, i + 32] = -1.0
        p64[i + 32, i] = 1.0
    p = np.zeros((128, 128), dtype=np.float32)
    p[0:64, 0:64] = p64
    p[64:128, 64:128] = p64
    return np.ascontiguousarray(p.T)  # lhsT = P^T


def kernel(x, Wq, bq, Wk, bk, Wv, bv, Wo, bo):
    x = np.asarray(x, dtype=np.float32)
    Wq, bq = np.asarray(Wq, np.float32), np.asarray(bq, np.float32)
    Wk, bk = np.asarray(Wk, np.float32), np.asarray(bk, np.float32)
    Wv, bv = np.asarray(Wv, np.float32), np.asarray(bv, np.float32)
    Wo, bo = np.asarray(Wo, np.float32), np.asarray(bo, np.float32)

    if "nc" not in _CACHE:
        _CACHE["nc"] = _build()
    nc = _CACHE["nc"]

    cosb, sinb = _rope_tables()
    rot = _rot_matrix()
    xt_b = [np.ascontiguousarray(x[b].T) for b in range(B)]  # [DM, S]
    wo_c = np.ascontiguousarray(Wo)

    in_maps = []
    for c in range(N_CORES):
        b, hg = divmod(c, HG)
        sl = slice(hg * GD, (hg + 1) * GD)
        in_maps.append(
            {
                "xt": xt_b[b],
                "wq": np.ascontiguousarray(Wq[:, sl]),
                "wk": np.ascontiguousarray(Wk[:, sl]),
                "wv": np.ascontiguousarray(Wv[:, sl]),
                "wo": wo_c,
                "bq2": np.ascontiguousarray(bq[sl].reshape(2, 128)),
                "bk2": np.ascontiguousarray(bk[sl].reshape(2, 128)),
                "bv": np.ascontiguousarray(bv[sl]),
                "cosb": cosb,
                "sinb": sinb,
                "rot": rot,
            }
        )

    res = bass_utils.run_bass_kernel_spmd(nc, in_maps, core_ids=list(range(N_CORES)))
    _CACHE["last_results"] = res

    # Block placement: core (b, hg), local head hl -> global head h = hg*4+hl,
    # lands at out[h//8, (h%8)*256 + b*128 : +128, :].
    out = np.empty((B, S, DM), dtype=np.float32)
    for c in range(N_CORES):
        b, hg = divmod(c, HG)
        ypt = res.results[c]["ypt"]  # [1024, 512]
        for hl in range(4):
            h = hg * 4 + hl
            b2 = h // 8
            s2 = (h % 8) * 256 + b * 128
            out[b2, s2:s2 + 128, :] = ypt[:, hl * 128:(hl + 1) * 128].T
    out += bo[None, None, :]
    return out
